# revision 8
# baseline (speedup 1.0000x reference)
"""MLA prefill attention (DeepSeek-style), tensor-parallel over heads on 8 TRN2 NeuronCores.

Reference computation (per head h, per batch b of 4 x 1024 tokens):
  kv_c   = k[:, 0, :512]                  # [N, 512] compressed latent (shared)
  k_nope = kv_c @ w_key[h].T              # [N, 128]
  k_full = concat(k_nope, k_rope)         # [N, 192]
  v_raw  = kv_c @ w_vo[h].T               # [N, 128]
  o      = softmax(causal(q_h @ k_full.T * SCALE)) @ v_raw

Sharding: 16 heads / 8 cores = 2 heads per core; kv_c replicated. No collectives.

Device kernel (per core, all matmuls bf16):
  Phase 1: k_nopeT [128d, N] per head; v_raw for BOTH heads per 128-token
    chunk in one 256-wide matmul (hides weight loads), stored interleaved as
    [v_h0 | 1 | v_h1 | 1] per chunk so each head's PV slice is contiguous
    [128,129] with a ones column.
  Phase 2: transposed-score flash attention: scoresT [k, q] = k_fullT.T @ qT,
    exp on ScalarE (softmax scale folded in; no max pass needed, scores are
    O(5) bounded), causal triangle masked multiplicatively on the first 128
    cols of diagonal chunks only (columns beyond the diagonal block are fully
    valid; trapezoid tiling skips the fully-masked region). PV uses probsT
    blocks as the STATIONARY operand and v_aug as moving, accumulating
    non-transposed o[q, dv] in PSUM with the softmax denominator in column
    128. Accumulators are bank-packed in pairs so two q-blocks pipeline.
    Epilogue: reciprocal + per-partition scaled copy on DVE, then DMA out.
"""

import os
import sys

sys.path.insert(0, "/opt/trn_rl_repo")

from contextlib import ExitStack

import numpy as np
import ml_dtypes

import concourse.bass as bass
import concourse.mybir as mybir
from concourse import bacc, tile
from concourse.bass_utils import run_bass_kernel_spmd

B, S, H, N = 4, 1024, 16, 4096
DN, DR, DV, R = 128, 64, 128, 512
SCALE = 0.07216878364870323
NCORES = 8
HPC = H // NCORES  # heads per core
P = 128
QBLK = 512
NRC = R // P  # 4 r-chunks
NBLK = 4      # kv column blocks (DMA pipelining granularity)
BCOLS = N // NBLK
DVA = DV + 1   # v | ones  -> rowsums fall out of PV
VCH = 2 * DVA  # combined both-heads v chunk stride [v0 | 1 | v1 | 1]
BF16 = mybir.dt.bfloat16
F32 = mybir.dt.float32
Exp = mybir.ActivationFunctionType.Exp

_CACHE: dict = {}


def _build():
    nc = bacc.Bacc("TRN2", target_bir_lowering=False, debug=False, num_devices=NCORES)

    qtn = nc.dram_tensor("qtn", [HPC, DN, N], BF16, kind="ExternalInput").ap()
    qtr = nc.dram_tensor("qtr", [HPC, DR, N], BF16, kind="ExternalInput").ap()
    kvt = nc.dram_tensor("kvt", [NRC, P, N], BF16, kind="ExternalInput").ap()
    krt = nc.dram_tensor("krt", [DR, N], BF16, kind="ExternalInput").ap()
    wkt = nc.dram_tensor("wkt", [HPC, NRC, P, DN], BF16, kind="ExternalInput").ap()
    # w_vo both heads combined: [c, 128r, 2*DV] = [w_vo[h0] | w_vo[h1]]
    wvt = nc.dram_tensor("wvt", [NRC, P, HPC * DV], BF16, kind="ExternalInput").ap()
    mskd = nc.dram_tensor("mskd", [P, P], BF16, kind="ExternalInput").ap()
    out = nc.dram_tensor("out", [HPC, N, DV], F32, kind="ExternalOutput").ap()

    with tile.TileContext(nc) as tc, ExitStack() as ctx:
        const = ctx.enter_context(tc.tile_pool(name="const", bufs=1))
        res = ctx.enter_context(tc.tile_pool(name="res", bufs=1))
        prs = ctx.enter_context(tc.tile_pool(name="prs", bufs=4))
        osb = ctx.enter_context(tc.tile_pool(name="osb", bufs=4))
        psA = ctx.enter_context(tc.tile_pool(name="psA", bufs=3, space="PSUM"))
        psO = ctx.enter_context(tc.tile_pool(name="psO", bufs=5, space="PSUM"))

        msk = const.tile([P, P], BF16)
        nc.sync.dma_start(msk[:], mskd[:])

        # kv_cT resident, split into column blocks so phase 1 can start
        # as soon as the first block of each r-chunk lands.
        kv_sb = [[None] * NBLK for _ in range(NRC)]
        for blk in range(NBLK):
            for c in range(NRC):
                t = res.tile([P, BCOLS], BF16, tag=f"kv{c}_{blk}", name=f"kv{c}_{blk}")
                nc.sync.dma_start(t[:], kvt[c, :, blk * BCOLS:(blk + 1) * BCOLS])
                kv_sb[c][blk] = t
        kr_sb = res.tile([DR, N], BF16)
        nc.sync.dma_start(kr_sb[:], krt[:])

        wk_sb, wv_sb = [], []
        for h in range(HPC):
            wkh = []
            for c in range(NRC):
                t = res.tile([P, DN], BF16, tag=f"wk{h}_{c}", name=f"wk{h}_{c}")
                nc.sync.dma_start(t[:], wkt[h, c])
                wkh.append(t)
            wk_sb.append(wkh)
        for c in range(NRC):
            t = res.tile([P, HPC * DV], BF16, tag=f"wv{c}", name=f"wv{c}")
            nc.sync.dma_start(t[:], wvt[c])
            wv_sb.append(t)

        kn_sb = [
            res.tile([P, N], BF16, tag=f"kn{h}", name=f"kn{h}") for h in range(HPC)
        ]
        # combined v_aug for both heads; chunk ki at [:, ki*VCH : (ki+1)*VCH]
        # = [v_h0(128) | 1 | v_h1(128) | 1]; memset pre-fills the ones cols.
        vcomb = res.tile([P, (N // P) * VCH], BF16)
        nc.gpsimd.memset(vcomb[:], 1.0)

        # ---- Phase 1 ----
        for blk in range(NBLK):
            # v for both heads: out [128n, 256] = kv_chunk.T @ [w_vo0|w_vo1]
            for ki in range(BCOLS // P):
                psv = psO.tile([P, HPC * DV], F32, tag="psO", name="vb")
                kg = blk * (BCOLS // P) + ki
                kl = slice(ki * P, (ki + 1) * P)
                for c in range(NRC):
                    nc.tensor.matmul(
                        psv[:], lhsT=kv_sb[c][blk][:, kl], rhs=wv_sb[c][:],
                        start=(c == 0), stop=(c == NRC - 1),
                    )
                # [128, 2, 128] -> strided dest blocks at 0 and DVA
                dst = vcomb[:, kg * VCH:(kg + 1) * VCH]
                nc.vector.tensor_copy(
                    dst.rearrange("p (h d) -> p h d", h=HPC)[:, :, 0:DV],
                    psv[:].rearrange("p (h d) -> p h d", h=HPC),
                )
            for h in range(HPC):
                for j in range(BCOLS // QBLK):
                    ps = psA.tile([P, QBLK], F32, tag="psA", name="knb")
                    js = slice(blk * BCOLS + j * QBLK, blk * BCOLS + (j + 1) * QBLK)
                    jl = slice(j * QBLK, (j + 1) * QBLK)
                    for c in range(NRC):
                        nc.tensor.matmul(
                            ps[:], lhsT=wk_sb[h][c][:], rhs=kv_sb[c][blk][:, jl],
                            start=(c == 0), stop=(c == NRC - 1),
                        )
                    nc.scalar.copy(kn_sb[h][:, js], ps[:])

        # q DMAs emitted after phase 1 so the kv blocks phase 1 depends on
        # win the DMA queues; q is only needed once phase 2 starts.
        qn_sb, qr_sb = [], []
        for h in range(HPC):
            t = res.tile([DN, N], BF16, tag=f"qn{h}", name=f"qn{h}")
            nc.sync.dma_start(t[:], qtn[h])
            qn_sb.append(t)
            t = res.tile([DR, N], BF16, tag=f"qr{h}", name=f"qr{h}")
            nc.sync.dma_start(t[:], qtr[h])
            qr_sb.append(t)

        # ---- Phase 2: attention ----
        for h in range(HPC):
            for b in range(B):
                q0 = b * S
                for qb in range(S // QBLK):
                    qs = qb * QBLK
                    nfull = qs // P
                    kis = nfull + QBLK // P
                    # two bank-packed accumulator pairs:
                    # [o_j4(128) | rs | o_j4+1(128) | rs]
                    ops = [
                        psO.tile([P, 2 * DVA], F32, tag="psO", name=f"opair{p_}")
                        for p_ in range(2)
                    ]
                    for ki in range(kis):
                        if ki < nfull:
                            j, qoff, w = -1, 0, QBLK
                        else:
                            j = ki - nfull
                            qoff = j * P
                            w = QBLK - qoff
                        qg = q0 + qs + qoff
                        kg = q0 + ki * P
                        ks = slice(kg, kg + P)
                        sc = psA.tile([P, QBLK], F32, tag="psA", name="sc")
                        nc.tensor.matmul(
                            sc[:, :w], lhsT=kn_sb[h][:, ks],
                            rhs=qn_sb[h][:, qg:qg + w], start=True, stop=False,
                        )
                        nc.tensor.matmul(
                            sc[:, :w], lhsT=kr_sb[:, ks],
                            rhs=qr_sb[h][:, qg:qg + w], start=False, stop=True,
                        )
                        pr = prs.tile([P, QBLK], BF16, tag="probs", name="pr")
                        nc.scalar.activation(pr[:, :w], sc[:, :w], Exp, scale=SCALE)
                        if ki >= nfull:
                            # only the leading 128 cols contain the triangle;
                            # all columns beyond the diagonal block are valid
                            nc.vector.tensor_mul(pr[:, 0:P], pr[:, 0:P], msk[:])
                        kidx = kg // P
                        va = vcomb[:, kidx * VCH + h * DVA:kidx * VCH + h * DVA + DVA]
                        for j4 in range(max(0, j), QBLK // P):
                            # start=True clears has_written for the WHOLE
                            # bank, so only the first write of each bank-packed
                            # pair may use it; the partner's first matmul
                            # overwrites via the already-cleared bits.
                            nc.tensor.matmul(
                                ops[j4 // 2][:, (j4 % 2) * DVA:(j4 % 2 + 1) * DVA],
                                lhsT=pr[:, j4 * P - qoff:(j4 + 1) * P - qoff],
                                rhs=va,
                                start=(ki == 0 and j4 % 2 == 0),
                                stop=(ki == nfull + j4),
                                skip_group_check=True,
                            )
                    for j4 in range(QBLK // P):
                        op = ops[j4 // 2]
                        off = (j4 % 2) * DVA
                        rec = osb.tile([P, 1], F32, tag="rec", name="rec")
                        nc.vector.reciprocal(rec[:], op[:, off + DV:off + DVA])
                        og = osb.tile([P, DV], F32, tag="og", name="og")
                        nc.vector.tensor_scalar_mul(og[:], op[:, off:off + DV], rec[:])
                        qg4 = q0 + qs + j4 * P
                        nc.sync.dma_start(out[h, qg4:qg4 + P, :], og[:])

    nc.compile()
    return nc


def _prep_inputs(q, k, w_key, w_vo):
    bf = ml_dtypes.bfloat16
    kv_c = np.ascontiguousarray(k[:, 0, :R])          # [N, 512]
    k_rope = np.ascontiguousarray(k[:, 0, R:])        # [N, 64]
    kvt = np.ascontiguousarray(
        kv_c.T.reshape(NRC, P, N).astype(bf))         # [4, 128, N]
    krt = np.ascontiguousarray(k_rope.T.astype(bf))   # [64, N]
    msk = np.triu(np.ones((P, P), np.float32)).astype(bf)  # msk[kl,t]=t>=kl

    in_maps = []
    for core in range(NCORES):
        hs = slice(core * HPC, (core + 1) * HPC)
        qh = q[:, hs, :]                              # [N, HPC, 192]
        qtn = np.ascontiguousarray(
            qh[:, :, :DN].transpose(1, 2, 0).astype(bf))   # [HPC, 128, N]
        qtr = np.ascontiguousarray(
            qh[:, :, DN:].transpose(1, 2, 0).astype(bf))   # [HPC, 64, N]
        # w_key[h]: [128d, 512r] -> w.T r-chunks: [HPC, 4, 128r, 128d]
        wkt = np.ascontiguousarray(
            w_key[hs].transpose(0, 2, 1).reshape(HPC, NRC, P, DN).astype(bf))
        # w_vo[hs]: [HPC, 128d, 512r] -> [c, 128r, HPC*128d] = [w0.T | w1.T]
        wvt = np.ascontiguousarray(
            w_vo[hs].transpose(2, 0, 1)               # [512r, HPC, 128d]
            .reshape(NRC, P, HPC * DV).astype(bf))
        in_maps.append({
            "qtn": qtn, "qtr": qtr, "kvt": kvt, "krt": krt,
            "wkt": wkt, "wvt": wvt, "mskd": msk,
        })
    return in_maps


def run(q, k, v, w_key, w_vo, trace=False, tmpdir=None):
    """Returns (output [N, H, 128] f32, BassKernelResults)."""
    if "nc" not in _CACHE:
        _CACHE["nc"] = _build()
    nc = _CACHE["nc"]
    in_maps = _prep_inputs(np.asarray(q), np.asarray(k),
                           np.asarray(w_key), np.asarray(w_vo))
    res = run_bass_kernel_spmd(
        nc, in_maps, core_ids=list(range(NCORES)), trace=trace, tmpdir=tmpdir
    )
    outs = [np.asarray(res.results[i]["out"], dtype=np.float32)
            for i in range(NCORES)]
    full = np.concatenate(outs, axis=0)               # [16, N, 128]
    return np.ascontiguousarray(full.transpose(1, 0, 2)), res


def kernel(q, k, v, w_key, w_vo):
    return run(q, k, v, w_key, w_vo)[0]
